# revision 7
# baseline (speedup 1.0000x reference)
"""Trainium2 Bass kernel for nn_KVEmbedding (embedding row-gather).

Problem: out[b, l, :] = table[indices[b, l], :]
  indices: (4096, 200) int64, values in [0, 1e6)
  table:   (1000000, 64) float32
  out:     (4096, 200, 64) float32

This environment reaches the 8 NeuronCores through an axon tunnel whose
host<->device link moves ~30-40 MB/s, half-duplex, shared across cores.
End-to-end time is therefore dominated by wire bytes, so the sharding
strategy is chosen to minimize them:

  host   - dedup the 819,200 lookups (~559k unique rows), fetch the unique
           rows once, round to bf16 (max rel err 2^-9 ~ 0.2%, far inside
           the 2e-2 gate), and shard rows *by compact position* across the
           8 cores (balanced split of the actual unique count).  Route each
           lookup to its owning core (the host-side stand-in for the
           all-to-all in the sharding hint, since inputs arrive via host).
  device - each core holds its [S, 64] bf16 shard and performs the real
           embedding lookup: ~102k indirect-DMA row gathers into SBUF,
           double-buffered, written back as a [R_CAP, 64] bf16 tensor.
  host   - un-permute the gathered rows into the (4096, 200, 64) f32 output.

Wire traffic per call: ~72 MB table shards + ~3 MB indices up, ~107 MB
gathered rows down  (vs ~2.5 GB for the replicated-table f32 baseline).

HW indirect-DMA semantics (validated empirically): ONE offset per partition
per instruction, each moving one contiguous 64-elem table row into that
partition.  So each gather instruction moves 128 rows (offsets = one column
of the idx tile); W gathers fill a wide SBUF buffer which is then written
out with a single large HWDGE DMA.  Double-buffered:

  Pool (SWDGE): Q indirect gathers, chunk c -> buffer (c//W)%2 col c%W
  SP   (HWDGE): idx load + Q/W writeouts of [128, W*64] to out DRAM

Capacity planning (input stats per spec: indices ~ uniform randint 1e6):
  expected unique rows  = 1e6*(1-exp(-0.8192)) ~ 559k; U_CAP = 566,272
  expected rows/core    = 102,400 (+-300);       R_CAP = 104,448
Lookups that overflow either capacity (essentially impossible for the spec
distribution, but possible for adversarial inputs) are patched on host, so
the kernel is correct for ANY input.
"""

import contextlib

import numpy as np
import ml_dtypes

import concourse.bass as bass
import concourse.mybir as mybir

B, L, D = 4096, 200, 64
N_CORES = 8
P = 128                # SBUF partitions
Q = 816                # gathered rows per partition = gather instructions
R_CAP = P * Q          # 104,448 lookups served per core
S = 70_784             # unique-row shard capacity per core
U_CAP = S * N_CORES    # 566,272 total unique-row capacity
W = 51                 # gather columns per writeout buffer (Q = 16*W)
NBUF = 2               # writeout buffers

BF16 = ml_dtypes.bfloat16

_state = None


def build():
    """Per-core gather kernel: out[r, :] = shard[idx[r], :] for r in [0, R_CAP)."""
    nwrite = Q // W
    assert nwrite * W == Q and nwrite % NBUF == 0
    nc = bass.Bass()
    idx = nc.dram_tensor("idx", [R_CAP], mybir.dt.int32, kind="ExternalInput")
    shard = nc.dram_tensor("shard", [S, D], mybir.dt.bfloat16, kind="ExternalInput")
    out = nc.dram_tensor("out", [R_CAP, D], mybir.dt.bfloat16, kind="ExternalOutput")

    idx_v = idx[:].rearrange("(p q) -> p q", p=P)          # [128, Q]
    out_v = out[:].rearrange("(p q) d -> p q d", p=P)      # [128, Q, 64]

    with contextlib.ExitStack() as ctx:
        idx_sb = ctx.enter_context(nc.sbuf_tensor([P, Q], mybir.dt.int32))
        bufs = [
            ctx.enter_context(
                nc.sbuf_tensor(f"buf{i}", [P, W * D], mybir.dt.bfloat16)
            )
            for i in range(NBUF)
        ]
        idx_sem = ctx.enter_context(nc.semaphore())
        gb_sems = [
            ctx.enter_context(nc.semaphore(name=f"gb_sem{i}")) for i in range(NBUF)
        ]
        wb_sems = [
            ctx.enter_context(nc.semaphore(name=f"wb_sem{i}")) for i in range(NBUF)
        ]
        block = ctx.enter_context(nc.Block())

        @block.sync
        def _(s):
            s.dma_start(idx_sb[:], idx_v).then_inc(idx_sem, 16)
            for wr in range(nwrite):
                b = wr % NBUF
                s.wait_ge(gb_sems[b], (wr // NBUF + 1) * W * 16)
                s.dma_start(out_v[:, wr * W:(wr + 1) * W, :], bufs[b][:]).then_inc(
                    wb_sems[b], 16
                )

        @block.gpsimd
        def _(gp):
            gp.wait_ge(idx_sem, 16)
            for c in range(Q):
                wr = c // W
                b = wr % NBUF
                j = c % W
                if j == 0 and wr >= NBUF:
                    gp.wait_ge(wb_sems[b], (wr // NBUF) * 16)
                gp.indirect_dma_start(
                    out=bufs[b][:, j * D:(j + 1) * D],
                    out_offset=None,
                    in_=shard[:],
                    in_offset=bass.IndirectOffsetOnAxis(
                        ap=idx_sb[:, c:c + 1], axis=0
                    ),
                ).then_inc(gb_sems[b], 16)

    return nc


def _get_runner():
    """Build the Bass module once and wrap it in a cached sharded jit."""
    global _state
    if _state is not None:
        return _state

    import jax
    import jax.numpy as jnp
    from jax.experimental.shard_map import shard_map
    from jax.sharding import Mesh, NamedSharding, PartitionSpec

    from concourse.bass2jax import (
        _bass_exec_p,
        install_neuronx_cc_hook,
        partition_id_tensor,
    )

    install_neuronx_cc_hook()
    nc = build()
    pid_name = nc.partition_id_tensor.name
    devices = jax.devices()[:N_CORES]
    mesh = Mesh(np.asarray(devices), ("core",))
    out_aval = jax.core.ShapedArray((R_CAP, D), BF16)

    def _body(idx, shard, zout):
        # zout is donation fodder: an output-shaped buffer the runtime reuses
        # for "out" (run_bass_via_pjrt ships host zeros for this; we make it
        # on-device so it never crosses the tunnel).
        outs = _bass_exec_p.bind(
            idx,
            shard,
            zout,
            partition_id_tensor(),
            out_avals=(out_aval,),
            in_names=("idx", "shard", "out", pid_name),
            out_names=("out",),
            lowering_input_output_aliases=(),
            sim_require_finite=True,
            sim_require_nnan=True,
            nc=nc,
        )
        return outs[0]

    fn = jax.jit(
        shard_map(
            _body,
            mesh=mesh,
            in_specs=(PartitionSpec("core"),) * 3,
            out_specs=PartitionSpec("core"),
            check_rep=False,
        ),
        donate_argnums=(2,),
    )
    zfn = jax.jit(
        lambda: jnp.zeros((N_CORES * R_CAP, D), BF16),
        out_shardings=NamedSharding(mesh, PartitionSpec("core")),
    )
    _state = (fn, zfn)
    return _state


def kernel(indices, table, dummy):
    fn, zfn = _get_runner()

    idx = np.ascontiguousarray(np.asarray(indices).reshape(-1)).astype(np.int32)
    n = idx.size
    table = np.asarray(table)

    # -- dedup + balanced compact-position sharding ---------------------------
    uniq, inv = np.unique(idx, return_inverse=True)
    inv = inv.astype(np.int64).ravel()
    n_u = uniq.size
    bnd = (n_u * np.arange(N_CORES + 1)) // N_CORES          # row split per core
    owner = np.searchsorted(bnd[1:], inv, side="right")      # in [0, 8)
    local = (inv - bnd[owner]).astype(np.int32)
    order = np.argsort(owner, kind="stable")
    counts = np.bincount(owner, minlength=N_CORES)
    starts = np.concatenate(([0], np.cumsum(counts)))

    # unique rows, bf16, laid out shard-contiguously
    urows = table[uniq].astype(BF16)                          # [n_u, 64]
    g = np.zeros((N_CORES * S, D), dtype=BF16)
    gi = np.zeros(N_CORES * R_CAP, dtype=np.int32)
    served = []
    for c in range(N_CORES):
        lc = min(int(bnd[c + 1] - bnd[c]), S)
        g[c * S:c * S + lc] = urows[bnd[c]:bnd[c] + lc]
        pos = order[starts[c]:starts[c + 1]]
        li = local[pos]
        if lc < bnd[c + 1] - bnd[c]:                          # shard overflow
            keep = li < S
            pos, li = pos[keep], li[keep]
        pos, li = pos[:R_CAP], li[:R_CAP]                     # count overflow
        gi[c * R_CAP:c * R_CAP + li.size] = li
        served.append(pos)

    # -- the on-device gather -------------------------------------------------
    og = np.asarray(fn(gi, g, zfn()))                         # [8*R_CAP, 64] bf16

    # -- un-permute into the full f32 output ----------------------------------
    res = np.empty((n, D), dtype=np.float32)
    n_served = 0
    for c in range(N_CORES):
        m = served[c].size
        n_served += m
        res[served[c]] = og[c * R_CAP:c * R_CAP + m].astype(np.float32)
    if n_served != n:                                         # host patch path
        mask = np.ones(n, dtype=bool)
        for c in range(N_CORES):
            mask[served[c]] = False
        rest = np.nonzero(mask)[0]
        res[rest] = table[idx[rest]].astype(np.float32)

    return res.reshape(np.asarray(indices).shape + (D,))


# revision 9
# speedup vs baseline: 1.1730x; 1.1730x over previous
"""Trainium2 Bass kernel for nn_KVEmbedding (embedding row-gather).

Problem: out[b, l, :] = table[indices[b, l], :]
  indices: (4096, 200) int64, values in [0, 1e6)
  table:   (1000000, 64) float32
  out:     (4096, 200, 64) float32

This environment reaches the 8 NeuronCores through an axon tunnel whose
host<->device link moves ~30-40 MB/s, half-duplex, shared across cores.
End-to-end time is therefore dominated by wire bytes, so the sharding
strategy minimizes them:

  host   - dedup the 819,200 lookups (~559k unique rows), round the unique
           rows ONCE to the e6m5 grid (max rel err 2^-6 = 1.5625%, inside
           the 2e-2 gate with margin; e6m5 values are exact in bf16), and
           shard rows by compact position across the 8 cores (balanced
           split of the actual unique count).  Route each lookup to its
           owning core (the host-side stand-in for the all-to-all in the
           sharding hint, since inputs arrive via host anyway).
  device - each core holds its [S, 64] shard and performs the real
           embedding lookup: ~102k indirect-DMA row gathers into SBUF,
           then the vector engine packs each gathered bf16 value into a
           12-bit e6m5 code (hi-byte plane + nibble plane), and the packed
           planes stream back.  Gather / encode / writeout are pipelined
           across engines with double buffering.
  host   - decode the 12-bit planes, un-permute into the (4096, 200, 64)
           f32 output.

Wire traffic per call: ~72 MB table shards + ~3 MB indices up, ~80 MB
packed rows down (vs ~2.5 GB for the replicated-table f32 baseline).

e6m5 code (12 bits): sign<<11 | (exp8-60)<<5 | m5, where exp8/m5 are the
bf16 fields.  Representable range 2^-67 .. 2^-4, which covers any
N(0, 0.01) table (the spec's fill) with astronomic margin; kernel() guards
the actual data range and falls back to a plain bf16-out kernel (lazily
compiled) for inputs outside it, so the kernel is correct for ANY input.
Lookups that overflow the capacity planning (U_CAP/R_CAP, sized ~7 sigma
above the spec distribution) are patched on host.

HW indirect-DMA semantics (validated empirically): ONE offset per
partition per instruction, each moving one contiguous 64-elem table row
into that partition; each gather instruction therefore moves 128 rows
(offsets = one column of the idx tile).

Engine pipeline per core (Q=816 gathers, W=51 per round, 16 rounds,
2 rotating buffers):
  gpsimd (SWDGE): indirect gathers into buf[b]
  vector:         buf[b] -> 12-bit codes -> hi-plane Hb[b] + nib-plane Nb[b]
  sync   (HWDGE): idx load + Hb/Nb writeouts to DRAM
"""

import contextlib

import numpy as np
import ml_dtypes

import concourse.bass as bass
import concourse.mybir as mybir

B, L, D = 4096, 200, 64
N_CORES = 8
P = 128                # SBUF partitions
Q = 816                # gathered rows per partition = gather instructions
R_CAP = P * Q          # 104,448 lookups served per core
S = 70_784             # unique-row shard capacity per core
U_CAP = S * N_CORES    # 566,272 total unique-row capacity
W = 51                 # gather columns per round (Q = 16*W)
NROUND = Q // W        # 16 writeout rounds
NBUF = 2               # rotating buffers

BF16 = ml_dtypes.bfloat16
EXP_LO, EXP_HI = 60, 123   # representable bf16 exponent window of e6m5

_state = None
_fallback = None


def build(packed=True):
    """Per-core gather(+pack) kernel over a [S, 64] u16 shard."""
    nc = bass.Bass()
    idx = nc.dram_tensor("idx", [R_CAP], mybir.dt.int32, kind="ExternalInput")
    shard = nc.dram_tensor("shard", [S, D], mybir.dt.uint16, kind="ExternalInput")
    if packed:
        out_h = nc.dram_tensor("out_h", [P, Q * 32], mybir.dt.uint16,
                               kind="ExternalOutput")
        out_n = nc.dram_tensor("out_n", [P, Q * 16], mybir.dt.uint16,
                               kind="ExternalOutput")
    else:
        out = nc.dram_tensor("out", [R_CAP, D], mybir.dt.uint16,
                             kind="ExternalOutput")
        out_v = out[:].rearrange("(p q) d -> p q d", p=P)  # [128, Q, 64]

    idx_v = idx[:].rearrange("(p q) -> p q", p=P)          # [128, Q]
    A = mybir.AluOpType
    VB = W * D             # 3264 values per partition per round

    with contextlib.ExitStack() as ctx:
        idx_sb = ctx.enter_context(nc.sbuf_tensor([P, Q], mybir.dt.int32))
        bufs = [
            ctx.enter_context(nc.sbuf_tensor(f"buf{i}", [P, VB], mybir.dt.uint16))
            for i in range(NBUF)
        ]
        if packed:
            t_sb = ctx.enter_context(nc.sbuf_tensor("enc_t", [P, VB], mybir.dt.uint16))
            ca = ctx.enter_context(nc.sbuf_tensor("enc_ca", [P, VB], mybir.dt.uint16))
            cb = ctx.enter_context(nc.sbuf_tensor("enc_cb", [P, VB], mybir.dt.uint16))
            code = ctx.enter_context(nc.sbuf_tensor("enc_c", [P, VB], mybir.dt.uint16))
            he = ctx.enter_context(nc.sbuf_tensor("enc_he", [P, VB // 2], mybir.dt.uint16))
            ho = ctx.enter_context(nc.sbuf_tensor("enc_ho", [P, VB // 2], mybir.dt.uint16))
            n0 = ctx.enter_context(nc.sbuf_tensor("enc_n0", [P, VB // 4], mybir.dt.uint16))
            n1 = ctx.enter_context(nc.sbuf_tensor("enc_n1", [P, VB // 4], mybir.dt.uint16))
            n2 = ctx.enter_context(nc.sbuf_tensor("enc_n2", [P, VB // 4], mybir.dt.uint16))
            n3 = ctx.enter_context(nc.sbuf_tensor("enc_n3", [P, VB // 4], mybir.dt.uint16))
            Hb = [
                ctx.enter_context(nc.sbuf_tensor(f"H{i}", [P, VB // 2], mybir.dt.uint16))
                for i in range(NBUF)
            ]
            Nb = [
                ctx.enter_context(nc.sbuf_tensor(f"N{i}", [P, VB // 4], mybir.dt.uint16))
                for i in range(NBUF)
            ]
        idx_sem = ctx.enter_context(nc.semaphore())
        gb_sems = [
            ctx.enter_context(nc.semaphore(name=f"gb_sem{i}")) for i in range(NBUF)
        ]
        enc_sems = [
            ctx.enter_context(nc.semaphore(name=f"enc_sem{i}")) for i in range(NBUF)
        ]
        wb_sems = [
            ctx.enter_context(nc.semaphore(name=f"wb_sem{i}")) for i in range(NBUF)
        ]
        block = ctx.enter_context(nc.Block())

        if packed:

            @block.sync
            def _(s):
                s.dma_start(idx_sb[:], idx_v).then_inc(idx_sem, 16)
                for wr in range(NROUND):
                    b = wr % NBUF
                    s.wait_ge(enc_sems[b], wr // NBUF + 1)
                    s.dma_start(
                        out_h[:, wr * (VB // 2):(wr + 1) * (VB // 2)], Hb[b][:]
                    ).then_inc(wb_sems[b], 16)
                    s.dma_start(
                        out_n[:, wr * (VB // 4):(wr + 1) * (VB // 4)], Nb[b][:]
                    ).then_inc(wb_sems[b], 16)

            @block.vector
            def _(v):
                for wr in range(NROUND):
                    b = wr % NBUF
                    v.wait_ge(gb_sems[b], (wr // NBUF + 1) * W * 16)
                    if wr >= NBUF:
                        v.wait_ge(wb_sems[b], (wr // NBUF) * 32)
                    buf = bufs[b]
                    # t = (y + 2) - 7680   (saturating u16 ALU; round-to-m5
                    # is a no-op here because the host pre-rounds to the
                    # e6m5 grid, but +2 keeps the device exact regardless)
                    v.tensor_scalar(t_sb[:], buf[:], 2, 7680, A.add, A.subtract)
                    # code12 = (t>>2)&0x7FF | sign<<11
                    v.tensor_scalar(ca[:], t_sb[:], 2, 0x7FF,
                                    A.logical_shift_right, A.bitwise_and)
                    v.tensor_scalar(cb[:], t_sb[:], 15, 11,
                                    A.logical_shift_right, A.logical_shift_left)
                    v.tensor_tensor(code[:], ca[:], cb[:], A.bitwise_or)
                    # hi-byte plane: H[k] = hi8(2k) | hi8(2k+1)<<8
                    v.tensor_scalar(he[:], code[:, 0::2], 4, None,
                                    A.logical_shift_right)
                    v.tensor_scalar(ho[:], code[:, 1::2], 4, 0xFF00,
                                    A.logical_shift_left, A.bitwise_and)
                    v.tensor_tensor(Hb[b][:], he[:], ho[:], A.bitwise_or)
                    # nibble plane: N[k] = n(4k)<<4|n(4k+1) | n(4k+2)<<12|n(4k+3)<<8
                    v.tensor_scalar(n0[:], code[:, 0::4], 0xF, 4,
                                    A.bitwise_and, A.logical_shift_left)
                    v.tensor_scalar(n1[:], code[:, 1::4], 0xF, None, A.bitwise_and)
                    v.tensor_scalar(n2[:], code[:, 2::4], 0xF, 12,
                                    A.bitwise_and, A.logical_shift_left)
                    v.tensor_scalar(n3[:], code[:, 3::4], 0xF, 8,
                                    A.bitwise_and, A.logical_shift_left)
                    v.tensor_tensor(n0[:], n0[:], n1[:], A.bitwise_or)
                    v.tensor_tensor(n2[:], n2[:], n3[:], A.bitwise_or)
                    v.tensor_tensor(Nb[b][:], n0[:], n2[:], A.bitwise_or).then_inc(
                        enc_sems[b], 1
                    )

        else:

            @block.sync
            def _(s):
                s.dma_start(idx_sb[:], idx_v).then_inc(idx_sem, 16)
                for wr in range(NROUND):
                    b = wr % NBUF
                    s.wait_ge(gb_sems[b], (wr // NBUF + 1) * W * 16)
                    s.dma_start(
                        out_v[:, wr * W:(wr + 1) * W, :], bufs[b][:]
                    ).then_inc(enc_sems[b], 16)

        @block.gpsimd
        def _(gp):
            gp.wait_ge(idx_sem, 16)
            for c in range(Q):
                wr = c // W
                b = wr % NBUF
                j = c % W
                if j == 0 and wr >= NBUF:
                    # buffer b free once the consumer is done with round wr-2
                    n_done = wr // NBUF
                    gp.wait_ge(enc_sems[b], n_done * (1 if packed else 16))
                gp.indirect_dma_start(
                    out=bufs[b][:, j * D:(j + 1) * D],
                    out_offset=None,
                    in_=shard[:],
                    in_offset=bass.IndirectOffsetOnAxis(
                        ap=idx_sb[:, c:c + 1], axis=0
                    ),
                ).then_inc(gb_sems[b], 16)

    return nc


def _make_runner(nc, out_specs_shapes):
    """Wrap a Bass module in a cached sharded jit (mirrors run_bass_via_pjrt's
    shard_map path, minus the per-call retrace and host-zero shipping)."""
    import jax
    import jax.numpy as jnp
    from jax.experimental.shard_map import shard_map
    from jax.sharding import Mesh, NamedSharding, PartitionSpec

    from concourse.bass2jax import (
        _bass_exec_p,
        install_neuronx_cc_hook,
        partition_id_tensor,
    )

    install_neuronx_cc_hook()
    pid_name = nc.partition_id_tensor.name
    devices = jax.devices()[:N_CORES]
    mesh = Mesh(np.asarray(devices), ("core",))
    out_names = tuple(n for n, _ in out_specs_shapes)
    out_avals = tuple(
        jax.core.ShapedArray(shape, np.uint16) for _, shape in out_specs_shapes
    )
    n_out = len(out_names)

    def _body(idx, shard, *zouts):
        # zouts are donation fodder: output-shaped buffers the runtime reuses
        # for the NEFF outputs (made on-device, never cross the tunnel).
        outs = _bass_exec_p.bind(
            idx,
            shard,
            *zouts,
            partition_id_tensor(),
            out_avals=out_avals,
            in_names=("idx", "shard") + out_names + (pid_name,),
            out_names=out_names,
            lowering_input_output_aliases=(),
            sim_require_finite=True,
            sim_require_nnan=True,
            nc=nc,
        )
        return tuple(outs)

    fn = jax.jit(
        shard_map(
            _body,
            mesh=mesh,
            in_specs=(PartitionSpec("core"),) * (2 + n_out),
            out_specs=(PartitionSpec("core"),) * n_out,
            check_rep=False,
        ),
        donate_argnums=tuple(range(2, 2 + n_out)),
    )
    sharding = NamedSharding(mesh, PartitionSpec("core"))
    zfn = jax.jit(
        lambda: tuple(
            jnp.zeros((N_CORES * shape[0],) + shape[1:], np.uint16)
            for _, shape in out_specs_shapes
        ),
        out_shardings=(sharding,) * n_out,
    )
    return fn, zfn


def _get_runner():
    global _state
    if _state is None:
        _state = _make_runner(
            build(packed=True),
            (("out_h", (P, Q * 32)), ("out_n", (P, Q * 16))),
        )
    return _state


def _get_fallback():
    global _fallback
    if _fallback is None:
        _fallback = _make_runner(build(packed=False), (("out", (R_CAP, D)),))
    return _fallback


def _round_to_e6m5(x32):
    """f32 -> nearest e6m5 value, returned as bf16 bit pattern (u16)."""
    u = np.ascontiguousarray(x32, dtype=np.float32).view(np.uint32)
    t = u + 0x1FFFF + ((u >> 18) & 1)          # RNE at mantissa bit 18
    return ((t >> 16) & np.uint32(0xFFFC)).astype(np.uint16)


def _decode_e6m5(oh, on):
    """Packed planes of one core -> [R_CAP, 64] f32 rows."""
    h8 = oh.view(np.uint8).reshape(P, Q * D)
    n8 = on.view(np.uint8).reshape(P, Q * D // 2)
    c = h8.astype(np.uint16) << 4
    c[:, 0::2] |= n8 >> 4
    c[:, 1::2] |= n8 & 0xF
    v = ((c & 0x7FF) << 2) + np.uint16(7680)
    v |= (c >> 11) << 15
    return v.view(BF16).astype(np.float32).reshape(R_CAP, D)


def kernel(indices, table, dummy):
    idx = np.ascontiguousarray(np.asarray(indices).reshape(-1)).astype(np.int32)
    n = idx.size
    table = np.asarray(table)

    # -- dedup + balanced compact-position sharding ---------------------------
    uniq, inv = np.unique(idx, return_inverse=True)
    inv = inv.astype(np.int64).ravel()
    n_u = uniq.size
    bnd = (n_u * np.arange(N_CORES + 1)) // N_CORES          # row split per core
    owner = np.searchsorted(bnd[1:], inv, side="right")      # in [0, 8)
    local = (inv - bnd[owner]).astype(np.int32)
    order = np.argsort(owner, kind="stable")
    counts = np.bincount(owner, minlength=N_CORES)
    starts = np.concatenate(([0], np.cumsum(counts)))

    # unique rows on the e6m5 grid, laid out shard-contiguously
    urows = _round_to_e6m5(table[uniq])                       # [n_u, 64] u16
    e = (urows >> 7) & np.uint16(0xFF)
    packable = bool(((e >= EXP_LO) & (e <= EXP_HI)).all())
    g = np.zeros((N_CORES * S, D), dtype=np.uint16)
    gi = np.zeros(N_CORES * R_CAP, dtype=np.int32)
    served = []
    for c in range(N_CORES):
        lc = min(int(bnd[c + 1] - bnd[c]), S)
        g[c * S:c * S + lc] = urows[bnd[c]:bnd[c] + lc]
        pos = order[starts[c]:starts[c + 1]]
        li = local[pos]
        if lc < bnd[c + 1] - bnd[c]:                          # shard overflow
            keep = li < S
            pos, li = pos[keep], li[keep]
        pos, li = pos[:R_CAP], li[:R_CAP]                     # count overflow
        gi[c * R_CAP:c * R_CAP + li.size] = li
        served.append(pos)

    # -- the on-device gather (+ 12-bit pack) ---------------------------------
    res = np.empty((n, D), dtype=np.float32)
    if packable:
        fn, zfn = _get_runner()
        oh, on = fn(gi, g, *zfn())
        oh, on = np.asarray(oh), np.asarray(on)
        for c in range(N_CORES):
            m = served[c].size
            rows = _decode_e6m5(oh[c * P:(c + 1) * P], on[c * P:(c + 1) * P])
            res[served[c]] = rows[:m]
    else:
        # data outside the e6m5 window: plain bf16 results (exact copy of the
        # bf16-rounded shard; host re-rounds g to bf16 for this path)
        urows_bf = np.asarray(table[uniq], dtype=np.float32).astype(BF16)
        g = np.zeros((N_CORES * S, D), dtype=np.uint16)
        for c in range(N_CORES):
            lc = min(int(bnd[c + 1] - bnd[c]), S)
            g[c * S:c * S + lc] = urows_bf[bnd[c]:bnd[c] + lc].view(np.uint16)
        fn, zfn = _get_fallback()
        (out,) = fn(gi, g, *zfn())
        og = np.asarray(out)
        for c in range(N_CORES):
            m = served[c].size
            rows = og[c * R_CAP:c * R_CAP + m].view(BF16).astype(np.float32)
            res[served[c]] = rows

    n_served = sum(s.size for s in served)
    if n_served != n:                                         # host patch path
        mask = np.ones(n, dtype=bool)
        for s in served:
            mask[s] = False
        rest = np.nonzero(mask)[0]
        res[rest] = table[idx[rest]].astype(np.float32)

    return res.reshape(np.asarray(indices).shape + (D,))


# revision 11
# speedup vs baseline: 1.5660x; 1.3351x over previous
"""Trainium2 Bass kernel for nn_KVEmbedding (embedding row-gather).

Problem: out[b, l, :] = table[indices[b, l], :]
  indices: (4096, 200) int64, values in [0, 1e6)
  table:   (1000000, 64) float32
  out:     (4096, 200, 64) float32

This environment reaches the 8 NeuronCores through an axon tunnel whose
host<->device link moves ~30-40 MB/s, half-duplex, shared across cores.
End-to-end time is therefore dominated by wire bytes, so the sharding
strategy minimizes them:

  host   - dedup the 819,200 lookups (~559k unique rows), round the unique
           rows ONCE to the e6m5 grid (max rel err 2^-6 = 1.5625%, inside
           the 2e-2 gate with margin; e6m5 values are exact in bf16), and
           shard rows by compact position across the 8 cores (balanced
           split of the actual unique count).  Route each lookup to its
           owning core (the host-side stand-in for the all-to-all in the
           sharding hint, since inputs arrive via host anyway).
  device - each core holds its [S, 64] shard and performs the real
           embedding lookup: ~102k indirect-DMA row gathers into SBUF,
           then the vector engine packs each gathered bf16 value into a
           12-bit e6m5 code (hi-byte plane + nibble plane), and the packed
           planes stream back.  Gather / encode / writeout are pipelined
           across engines with double buffering.
  host   - decode the 12-bit planes, un-permute into the (4096, 200, 64)
           f32 output.

Wire traffic per call: ~72 MB table shards + ~3 MB indices up, ~80 MB
packed rows down (vs ~2.5 GB for the replicated-table f32 baseline).

e6m5 code (12 bits): sign<<11 | (exp8-60)<<5 | m5, where exp8/m5 are the
bf16 fields.  Representable range 2^-67 .. 2^-4, which covers any
N(0, 0.01) table (the spec's fill) with astronomic margin; kernel() guards
the actual data range and falls back to a plain bf16-out kernel (lazily
compiled) for inputs outside it, so the kernel is correct for ANY input.
Lookups that overflow the capacity planning (U_CAP/R_CAP, sized ~7 sigma
above the spec distribution) are patched on host.

HW indirect-DMA semantics (validated empirically): ONE offset per
partition per instruction, each moving one contiguous 64-elem table row
into that partition; each gather instruction therefore moves 128 rows
(offsets = one column of the idx tile).

Engine pipeline per core (Q=816 gathers, W=51 per round, 16 rounds,
2 rotating buffers):
  gpsimd (SWDGE): indirect gathers into buf[b]
  vector:         buf[b] -> 12-bit codes -> hi-plane Hb[b] + nib-plane Nb[b]
  sync   (HWDGE): idx load + Hb/Nb writeouts to DRAM
"""

import contextlib

import numpy as np
import ml_dtypes

import concourse.bass as bass
import concourse.mybir as mybir

B, L, D = 4096, 200, 64
N_CORES = 8
P = 128                # SBUF partitions
Q = 816                # gathered rows per partition = gather instructions
R_CAP = P * Q          # 104,448 lookups served per core
S = 70_784             # unique-row shard capacity per core
U_CAP = S * N_CORES    # 566,272 total unique-row capacity
W = 51                 # gather columns per round (Q = 16*W)
NROUND = Q // W        # 16 writeout rounds
NBUF = 2               # rotating buffers

BF16 = ml_dtypes.bfloat16
EXP_LO, EXP_HI = 60, 123   # representable bf16 exponent window of e6m5

_state = None
_fallback = None


def build(packed=True):
    """Per-core gather(+pack) kernel over a [S, 64] u16 shard."""
    nc = bass.Bass()
    idx = nc.dram_tensor("idx", [R_CAP], mybir.dt.int32, kind="ExternalInput")
    shard = nc.dram_tensor("shard", [S, D], mybir.dt.uint16, kind="ExternalInput")
    if packed:
        out_h = nc.dram_tensor("out_h", [P, Q * 32], mybir.dt.uint16,
                               kind="ExternalOutput")
        out_n = nc.dram_tensor("out_n", [P, Q * 16], mybir.dt.uint16,
                               kind="ExternalOutput")
    else:
        out = nc.dram_tensor("out", [R_CAP, D], mybir.dt.uint16,
                             kind="ExternalOutput")
        out_v = out[:].rearrange("(p q) d -> p q d", p=P)  # [128, Q, 64]

    idx_v = idx[:].rearrange("(p q) -> p q", p=P)          # [128, Q]
    A = mybir.AluOpType
    VB = W * D             # 3264 values per partition per round

    with contextlib.ExitStack() as ctx:
        idx_sb = ctx.enter_context(nc.sbuf_tensor([P, Q], mybir.dt.int32))
        bufs = [
            ctx.enter_context(nc.sbuf_tensor(f"buf{i}", [P, VB], mybir.dt.uint16))
            for i in range(NBUF)
        ]
        if packed:
            t_sb = ctx.enter_context(nc.sbuf_tensor("enc_t", [P, VB], mybir.dt.uint16))
            ca = ctx.enter_context(nc.sbuf_tensor("enc_ca", [P, VB], mybir.dt.uint16))
            cb = ctx.enter_context(nc.sbuf_tensor("enc_cb", [P, VB], mybir.dt.uint16))
            code = ctx.enter_context(nc.sbuf_tensor("enc_c", [P, VB], mybir.dt.uint16))
            he = ctx.enter_context(nc.sbuf_tensor("enc_he", [P, VB // 2], mybir.dt.uint16))
            ho = ctx.enter_context(nc.sbuf_tensor("enc_ho", [P, VB // 2], mybir.dt.uint16))
            n0 = ctx.enter_context(nc.sbuf_tensor("enc_n0", [P, VB // 4], mybir.dt.uint16))
            n1 = ctx.enter_context(nc.sbuf_tensor("enc_n1", [P, VB // 4], mybir.dt.uint16))
            n2 = ctx.enter_context(nc.sbuf_tensor("enc_n2", [P, VB // 4], mybir.dt.uint16))
            n3 = ctx.enter_context(nc.sbuf_tensor("enc_n3", [P, VB // 4], mybir.dt.uint16))
            Hb = [
                ctx.enter_context(nc.sbuf_tensor(f"H{i}", [P, VB // 2], mybir.dt.uint16))
                for i in range(NBUF)
            ]
            Nb = [
                ctx.enter_context(nc.sbuf_tensor(f"N{i}", [P, VB // 4], mybir.dt.uint16))
                for i in range(NBUF)
            ]
        idx_sem = ctx.enter_context(nc.semaphore())
        gb_sems = [
            ctx.enter_context(nc.semaphore(name=f"gb_sem{i}")) for i in range(NBUF)
        ]
        enc_sems = [
            ctx.enter_context(nc.semaphore(name=f"enc_sem{i}")) for i in range(NBUF)
        ]
        wb_sems = [
            ctx.enter_context(nc.semaphore(name=f"wb_sem{i}")) for i in range(NBUF)
        ]
        block = ctx.enter_context(nc.Block())

        if packed:

            @block.sync
            def _(s):
                s.dma_start(idx_sb[:], idx_v).then_inc(idx_sem, 16)
                for wr in range(NROUND):
                    b = wr % NBUF
                    s.wait_ge(enc_sems[b], wr // NBUF + 1)
                    s.dma_start(
                        out_h[:, wr * (VB // 2):(wr + 1) * (VB // 2)], Hb[b][:]
                    ).then_inc(wb_sems[b], 16)
                    s.dma_start(
                        out_n[:, wr * (VB // 4):(wr + 1) * (VB // 4)], Nb[b][:]
                    ).then_inc(wb_sems[b], 16)

            @block.vector
            def _(v):
                for wr in range(NROUND):
                    b = wr % NBUF
                    v.wait_ge(gb_sems[b], (wr // NBUF + 1) * W * 16)
                    if wr >= NBUF:
                        v.wait_ge(wb_sems[b], (wr // NBUF) * 32)
                    buf = bufs[b]
                    # t = (y + 2) - 7680   (saturating u16 ALU; round-to-m5
                    # is a no-op here because the host pre-rounds to the
                    # e6m5 grid, but +2 keeps the device exact regardless)
                    v.tensor_scalar(t_sb[:], buf[:], 2, 7680, A.add, A.subtract)
                    # code12 = (t>>2)&0x7FF | sign<<11
                    v.tensor_scalar(ca[:], t_sb[:], 2, 0x7FF,
                                    A.logical_shift_right, A.bitwise_and)
                    v.tensor_scalar(cb[:], t_sb[:], 15, 11,
                                    A.logical_shift_right, A.logical_shift_left)
                    v.tensor_tensor(code[:], ca[:], cb[:], A.bitwise_or)
                    # hi-byte plane: H[k] = hi8(2k) | hi8(2k+1)<<8
                    v.tensor_scalar(he[:], code[:, 0::2], 4, None,
                                    A.logical_shift_right)
                    v.tensor_scalar(ho[:], code[:, 1::2], 4, 0xFF00,
                                    A.logical_shift_left, A.bitwise_and)
                    v.tensor_tensor(Hb[b][:], he[:], ho[:], A.bitwise_or)
                    # nibble plane: N[k] = n(4k)<<4|n(4k+1) | n(4k+2)<<12|n(4k+3)<<8
                    v.tensor_scalar(n0[:], code[:, 0::4], 0xF, 4,
                                    A.bitwise_and, A.logical_shift_left)
                    v.tensor_scalar(n1[:], code[:, 1::4], 0xF, None, A.bitwise_and)
                    v.tensor_scalar(n2[:], code[:, 2::4], 0xF, 12,
                                    A.bitwise_and, A.logical_shift_left)
                    v.tensor_scalar(n3[:], code[:, 3::4], 0xF, 8,
                                    A.bitwise_and, A.logical_shift_left)
                    v.tensor_tensor(n0[:], n0[:], n1[:], A.bitwise_or)
                    v.tensor_tensor(n2[:], n2[:], n3[:], A.bitwise_or)
                    v.tensor_tensor(Nb[b][:], n0[:], n2[:], A.bitwise_or).then_inc(
                        enc_sems[b], 1
                    )

        else:

            @block.sync
            def _(s):
                s.dma_start(idx_sb[:], idx_v).then_inc(idx_sem, 16)
                for wr in range(NROUND):
                    b = wr % NBUF
                    s.wait_ge(gb_sems[b], (wr // NBUF + 1) * W * 16)
                    s.dma_start(
                        out_v[:, wr * W:(wr + 1) * W, :], bufs[b][:]
                    ).then_inc(enc_sems[b], 16)

        @block.gpsimd
        def _(gp):
            gp.wait_ge(idx_sem, 16)
            for c in range(Q):
                wr = c // W
                b = wr % NBUF
                j = c % W
                if j == 0 and wr >= NBUF:
                    # buffer b free once the consumer is done with round wr-2
                    n_done = wr // NBUF
                    gp.wait_ge(enc_sems[b], n_done * (1 if packed else 16))
                gp.indirect_dma_start(
                    out=bufs[b][:, j * D:(j + 1) * D],
                    out_offset=None,
                    in_=shard[:],
                    in_offset=bass.IndirectOffsetOnAxis(
                        ap=idx_sb[:, c:c + 1], axis=0
                    ),
                ).then_inc(gb_sems[b], 16)

    return nc


def _make_runner(nc, out_specs_shapes):
    """Wrap a Bass module in a cached sharded jit (mirrors run_bass_via_pjrt's
    shard_map path, minus the per-call retrace and host-zero shipping)."""
    import jax
    import jax.numpy as jnp
    from jax.experimental.shard_map import shard_map
    from jax.sharding import Mesh, NamedSharding, PartitionSpec

    from concourse.bass2jax import (
        _bass_exec_p,
        install_neuronx_cc_hook,
        partition_id_tensor,
    )

    install_neuronx_cc_hook()
    pid_name = nc.partition_id_tensor.name
    devices = jax.devices()[:N_CORES]
    mesh = Mesh(np.asarray(devices), ("core",))
    out_names = tuple(n for n, _ in out_specs_shapes)
    out_avals = tuple(
        jax.core.ShapedArray(shape, np.uint16) for _, shape in out_specs_shapes
    )
    n_out = len(out_names)

    def _body(idx, shard, *zouts):
        # zouts are donation fodder: output-shaped buffers the runtime reuses
        # for the NEFF outputs (made on-device, never cross the tunnel).
        outs = _bass_exec_p.bind(
            idx,
            shard,
            *zouts,
            partition_id_tensor(),
            out_avals=out_avals,
            in_names=("idx", "shard") + out_names + (pid_name,),
            out_names=out_names,
            lowering_input_output_aliases=(),
            sim_require_finite=True,
            sim_require_nnan=True,
            nc=nc,
        )
        return tuple(outs)

    fn = jax.jit(
        shard_map(
            _body,
            mesh=mesh,
            in_specs=(PartitionSpec("core"),) * (2 + n_out),
            out_specs=(PartitionSpec("core"),) * n_out,
            check_rep=False,
        ),
        donate_argnums=tuple(range(2, 2 + n_out)),
    )
    sharding = NamedSharding(mesh, PartitionSpec("core"))
    zfn = jax.jit(
        lambda: tuple(
            jnp.zeros((N_CORES * shape[0],) + shape[1:], np.uint16)
            for _, shape in out_specs_shapes
        ),
        out_shardings=(sharding,) * n_out,
    )
    return {"fn": fn, "zfn": zfn, "devices": devices, "sharding": sharding,
            "zprev": None}


def _get_runner():
    global _state
    if _state is None:
        _state = _make_runner(
            build(packed=True),
            (("out_h", (P, Q * 32)), ("out_n", (P, Q * 16))),
        )
    return _state


def _get_fallback():
    global _fallback
    if _fallback is None:
        _fallback = _make_runner(build(packed=False), (("out", (R_CAP, D)),))
    return _fallback


def _round_to_e6m5(x32):
    """f32 -> nearest e6m5 value, returned as bf16 bit pattern (u16)."""
    u = np.ascontiguousarray(x32, dtype=np.float32).view(np.uint32)
    t = u + 0x1FFFF + ((u >> 18) & 1)          # RNE at mantissa bit 18
    return ((t >> 16) & np.uint32(0xFFFC)).astype(np.uint16)


def _decode_e6m5(oh, on):
    """Packed planes of one core -> [R_CAP, 64] f32 rows."""
    h8 = oh.view(np.uint8).reshape(P, Q * D)
    n8 = on.view(np.uint8).reshape(P, Q * D // 2)
    c = h8.astype(np.uint16) << 4
    c[:, 0::2] |= n8 >> 4
    c[:, 1::2] |= n8 & 0xF
    v = ((c & 0x7FF) << 2) + np.uint16(7680)
    v |= (c >> 11) << 15
    return v.view(BF16).astype(np.float32).reshape(R_CAP, D)


def _shards_by_core(arr, devices):
    """Per-device host fetches of a sharded array, ordered core 0..7."""
    by_dev = {sh.device: sh.data for sh in arr.addressable_shards}
    return [by_dev[d] for d in devices]


def kernel(indices, table, dummy):
    import jax
    from concurrent.futures import ThreadPoolExecutor

    st = _get_runner()
    idx = np.ascontiguousarray(np.asarray(indices).reshape(-1)).astype(np.int32)
    n = idx.size
    table = np.asarray(table)

    # -- dedup --------------------------------------------------------------
    uniq, inv = np.unique(idx, return_inverse=True)
    inv = inv.astype(np.int64).ravel()
    n_u = uniq.size
    bnd = (n_u * np.arange(N_CORES + 1)) // N_CORES          # row split per core
    lens = np.minimum(np.diff(bnd), S).astype(np.int64)

    # -- per-core shard build + async upload (overlaps routing below) --------
    urows_parts = []
    g_parts = []
    for c in range(N_CORES):
        rows = _round_to_e6m5(table[uniq[bnd[c]:bnd[c] + lens[c]]])
        urows_parts.append(rows)
        part = np.zeros((S, D), dtype=np.uint16)
        part[:lens[c]] = rows
        g_parts.append(jax.device_put(part, st["devices"][c]))  # async
    packable = all(
        bool((((r >> 7) & np.uint16(0xFF)) >= EXP_LO).all()
             and (((r >> 7) & np.uint16(0xFF)) <= EXP_HI).all())
        for r in urows_parts
    )
    g = jax.make_array_from_single_device_arrays(
        (N_CORES * S, D), st["sharding"], g_parts
    )

    # -- route lookups to owning cores (host stand-in for the all-to-all) ----
    owner = np.searchsorted(bnd[1:], inv, side="right")      # in [0, 8)
    local = (inv - bnd[owner]).astype(np.int32)
    order = np.argsort(owner, kind="stable")
    counts = np.bincount(owner, minlength=N_CORES)
    starts = np.concatenate(([0], np.cumsum(counts)))
    gi = np.zeros(N_CORES * R_CAP, dtype=np.int32)
    served = []
    for c in range(N_CORES):
        pos = order[starts[c]:starts[c + 1]]
        li = local[pos]
        if lens[c] < bnd[c + 1] - bnd[c]:                     # shard overflow
            keep = li < S
            pos, li = pos[keep], li[keep]
        pos, li = pos[:R_CAP], li[:R_CAP]                     # count overflow
        gi[c * R_CAP:c * R_CAP + li.size] = li
        served.append(pos)

    # -- the on-device gather (+ 12-bit pack) --------------------------------
    res = np.empty((n, D), dtype=np.float32)
    if packable:
        z = st["zprev"] if st["zprev"] is not None else st["zfn"]()
        st["zprev"] = None
        oh, on = st["fn"](gi, g, *z)
        st["zprev"] = (oh, on)  # donation fodder for the next call
        hs = _shards_by_core(oh, st["devices"])
        ns = _shards_by_core(on, st["devices"])
        # pipeline: fetch core c+1 over the wire while decoding/scattering c
        with ThreadPoolExecutor(2) as ex:
            futs = [
                ex.submit(lambda h, m: (np.asarray(h), np.asarray(m)), h, m)
                for h, m in zip(hs, ns)
            ]
            for c in range(N_CORES):
                oh_c, on_c = futs[c].result()
                rows = _decode_e6m5(oh_c, on_c)
                res[served[c]] = rows[:served[c].size]
    else:
        # data outside the e6m5 window: plain bf16 results (exact copy of the
        # bf16-rounded shard); lazily-compiled fallback, correct for ANY input
        fb = _get_fallback()
        urows_bf = np.asarray(table[uniq], dtype=np.float32).astype(BF16)
        gb = np.zeros((N_CORES * S, D), dtype=np.uint16)
        for c in range(N_CORES):
            gb[c * S:c * S + lens[c]] = (
                urows_bf[bnd[c]:bnd[c] + lens[c]].view(np.uint16)
            )
        (out,) = fb["fn"](gi, gb, *fb["zfn"]())
        og = np.asarray(out)
        for c in range(N_CORES):
            m = served[c].size
            res[served[c]] = (
                og[c * R_CAP:c * R_CAP + m].view(BF16).astype(np.float32)
            )

    n_served = sum(s.size for s in served)
    if n_served != n:                                         # host patch path
        mask = np.ones(n, dtype=bool)
        for s in served:
            mask[s] = False
        rest = np.nonzero(mask)[0]
        res[rest] = table[idx[rest]].astype(np.float32)

    return res.reshape(np.asarray(indices).shape + (D,))
